# revision 18
# baseline (speedup 1.0000x reference)
"""Single-head attention on 8 TRN2 NeuronCores (Bass/Tile).

Problem: x [4, 4096, 1024] f32; Wq/Wk/Wv [1024, 64]; bq/bk/bv [64].
  Q = x@Wq + bq; K = x@Wk + bk; V = x@Wv + bv
  out = softmax(Q K^T / 8) V        -> [4, 4096, 64]

Sharding: 8 cores = 4 batches x 2 query-halves. Every core gets its
batch's x pre-rotated (np.roll) on the host so its 2048 query rows are
always rows 0:2048 -> all cores run one identical static graph
(attention is permutation-invariant over keys). The host pre-permutes
x into the exact [chunk, partition, dchunk, s] SBUF layout (contiguous
8KB-per-partition DMA descriptors), pre-casts to bf16, and folds the
1/sqrt(64) score scale into Wq/bq. The device returns the UNNORMALIZED
attention output transposed ([64 h | 1 sums row] x 2048 queries); the
host divides by the sums row and transposes during the gather -- that
removes 16 PE transposes, the vector-engine normalize, and a
256B-run-strided output DMA from the device critical path.

Per-core kernel, v5:
- Scores for a key-tile PAIR run as two CONCURRENT row-tiled matmuls
  (h=64 contraction uses half the PE rows each; HW-measured ~2x).
  Even key tiles' K is DMA partition-shifted to rows 0:64 of KT;
  [Wq|Wq]-style duplication puts Q on both partition halves of QT.
- Q projections are col-tiled: one pass computes two 512-query chunks
  concurrently on the two PE column halves (64-wide Wq each), then two
  small DMAs mirror each chunk to the other partition half.
- Exp of the [128, 1024] score pair: scalar-engine ACTIVATE for most
  iterations; a share runs as Schraudolph fast-exp on the vector
  engine (TENSOR_SCALAR mult+add into int16, bit-viewed as bf16;
  HW-verified exact round-to-nearest, ~2% elementwise, scale-invariant
  under softmax).
- Software pipeline: iteration i emits scores(i)+exp(i); the PV
  accumulations of iteration i-2 follow, so neither exp engine ever
  waits on PV and the PE never waits on the current exp.
- Iteration order is PAIR-MAJOR over each half's two 512-query
  windows: pair p serves window 0 then window 1 before advancing, so
  the x-chunk DMA arrival window (first ~34us) overlaps 32 attention
  iterations instead of 16. KV chunk c drips in four pieces across
  iterations 4c-4..4c-1, finishing just before pair 2c needs it.
- V natural tiles via PE transposes; a ones column makes exp row-sums
  fall out of the PV matmul for free (row 64 of outT).
"""

import ml_dtypes
import numpy as np

import concourse.bass as bass
import concourse.mybir as mybir
import concourse.tile as tile
from concourse import bacc
from concourse.bass_utils import run_bass_kernel_spmd
from concourse.masks import make_identity

P = 128
D = 1024
DC = D // P  # 8 contraction chunks
S = 4096
SQ = 2048  # query rows per core
H = 64
NSC = S // 512  # 8 s-chunks of 512
NKT = S // P  # 32 key tiles of 128
NPAIR = NKT // 2  # 16 key-tile pairs
F32 = mybir.dt.float32
BF16 = mybir.dt.bfloat16
I16 = mybir.dt.int16
NP_BF16 = ml_dtypes.bfloat16

FE_SCALE = 128.0 / float(np.log(2.0))
FE_BIAS = 127.0 * 128.0 - 6.0

_NC_CACHE = {}


def build_core_graph():
    nc = bacc.Bacc(None, target_bir_lowering=False, debug=False)

    xt_h = nc.dram_tensor("xt", [NSC, P, DC, 512], BF16, kind="ExternalInput")
    wvk_h = nc.dram_tensor("wvk", [P, DC, P], BF16, kind="ExternalInput")
    wq_h = nc.dram_tensor("wq", [P, DC, H], BF16, kind="ExternalInput")
    b6_h = nc.dram_tensor("b6", [P, 3], F32, kind="ExternalInput")
    out2_h = nc.dram_tensor("out2", [H + 1, SQ], F32, kind="ExternalOutput")

    with tile.TileContext(nc) as tc:
        with (
            tc.tile_pool(name="const", bufs=1) as const,
            tc.tile_pool(name="xtp", bufs=8) as xtp,
            tc.tile_pool(name="expp", bufs=3) as expp,
            tc.tile_pool(name="otp", bufs=2) as otp,
            tc.tile_pool(name="pst", bufs=2, space="PSUM") as pst,
            tc.tile_pool(name="pkv", bufs=1, space="PSUM") as pkv,
            tc.tile_pool(name="pwork", bufs=1, space="PSUM") as pwork,
            tc.tile_pool(name="pout", bufs=2, space="PSUM") as pout,
        ):
            # ---- constants / persistent buffers ----
            wvk_sb = const.tile([P, DC, P], BF16, name="wvk_sb")
            wq_sb = const.tile([P, DC, H], BF16, name="wq_sb")
            b6_sb = const.tile([P, 3], F32, name="b6_sb")
            ident_b = const.tile([P, P], BF16, name="ident_b")
            KT = const.tile([P, S], BF16, name="KT")
            QT = const.tile([P, SQ], BF16, name="QT")
            # VT2: per key-tile PAIR, even kt's V^T on rows 0:64 and odd
            # kt's on 64:128 -> ONE [128,128] PE transpose yields BOTH
            # natural V tiles. VTst stages odd kts for the partition-shift
            # DMA (DVE cannot write across partition halves).
            VT2 = const.tile([P, S // 2], BF16, name="VT2")
            VTst = const.tile([H, NSC, 2, P], BF16, name="VTst")
            Vn = const.tile([P, NKT, H + 1], BF16, name="Vn")
            warm = const.tile([P, 3], F32, name="warm")

            nc.sync.dma_start(wvk_sb[:], wvk_h[:, :, :])
            nc.sync.dma_start(wq_sb[:], wq_h[:, :, :])
            nc.sync.dma_start(b6_sb[:], b6_h[:, :])
            make_identity(nc, ident_b[:])
            nc.gpsimd.memset(Vn[:, :, H : H + 1], 1.0)
            nc.scalar.activation(warm[:], b6_sb[:], mybir.ActivationFunctionType.Exp)
            # Dummy matmuls bridge the ~13us DMA lead-in so the HAM clock
            # gate stays released (1.2 -> 2.4 GHz) when real work arrives.
            # Rotate across the bank's four 128-col slices: consecutive
            # dummies hit disjoint psum regions, so they pipeline at the
            # ~56ns streaming rate instead of serializing on the WAW drain.
            wps = pkv.tile([P, 512], F32, tag="kv", name="warm_ps")
            for i in range(48):
                s = (i % 4) * P
                nc.tensor.matmul(
                    wps[:, s : s + P], ident_b[:], ident_b[:],
                    start=True, stop=True,
                )

            def load_chunk(sc):
                xtile = xtp.tile([P, DC, 512], BF16, name="xtile")
                nc.scalar.dma_start(xtile[:], xt_h[sc])
                return xtile

            def kv_mms(sc, xtile, lo, hi):
                sl = slice(sc * 512, (sc + 1) * 512)
                if lo == 0:
                    kv_mms.ps[sc] = pkv.tile(
                        [P, 512], F32, tag="kv", name=f"kvps{sc}"
                    )
                ps = kv_mms.ps[sc]
                first = None
                for dc in range(lo, hi):
                    h = nc.tensor.matmul(
                        ps[:], wvk_sb[:, dc, :], xtile[:, dc, :],
                        start=(dc == 0), stop=(dc == DC - 1),
                    )
                    first = first or h
                if hi == DC:
                    psv = ps[0:H, :].rearrange("p (b k) -> p b k", k=P)
                    # even kts (psum col-blocks 0,2) -> VT2 rows 0:64
                    nc.vector.tensor_scalar_add(
                        VT2[0:H, sc * 256 : (sc + 1) * 256].rearrange(
                            "p (b k) -> p b k", k=P),
                        psv[:, 0::2], b6_sb[0:H, 2:3],
                    )
                    # odd kts -> staging, then partition-shift DMA to rows 64:128
                    nc.vector.tensor_scalar_add(
                        VTst[:, sc], psv[:, 1::2], b6_sb[0:H, 2:3]
                    )
                    nc.sync.dma_start(
                        VT2[H:P, sc * 256 : (sc + 1) * 256].rearrange(
                            "p (b k) -> p b k", k=P),
                        VTst[:, sc],
                    )
                    nc.vector.tensor_scalar_add(KT[H:P, sl], ps[H:P, :], b6_sb[H:P, 1:2])
                    nc.sync.dma_start(
                        KT[0:H, sl].rearrange("p (b k) -> p b k", k=P)[:, 0::2],
                        KT[H:P, sl].rearrange("p (b k) -> p b k", k=P)[:, 0::2],
                    )
                return first
            kv_mms.ps = {}

            def v_trans(sc, t0, t1):
                # one [128,128] transpose per key-tile PAIR
                first = None
                for t in range(t0, t1):
                    pr = sc * 2 + t
                    psl = slice(pr * P, (pr + 1) * P)
                    tp = pwork.tile([P, P], BF16, tag="work", name=f"vtp{pr}")
                    h = nc.tensor.transpose(tp[:], VT2[:, psl], ident_b[:])
                    first = first or h
                    nc.vector.tensor_copy(
                        Vn[:, 2 * pr : 2 * pr + 2, 0:H],
                        tp[:].rearrange("p (b k) -> p b k", k=H),
                    )
                return first

            def q_pass2(se, so, xte, xto):
                """Col-tiled [Wq] pass: chunk se -> psum rows 0:64, chunk so
                -> rows 64:128, concurrently; then mirror each to the other
                partition half of QT via DMA."""
                ps = pkv.tile([P, 512], F32, tag="kv", name=f"qps{se}")
                first = None
                for dc in range(DC):
                    h = nc.tensor.matmul(
                        ps[0:H, :], wq_sb[:, dc, :], xte[:, dc, :],
                        start=(dc == 0), stop=(dc == DC - 1),
                        tile_position=(0, 0), skip_group_check=True,
                    )
                    first = first or h
                    nc.tensor.matmul(
                        ps[H:P, :], wq_sb[:, dc, :], xto[:, dc, :],
                        start=(dc == 0), stop=(dc == DC - 1),
                        tile_position=(0, 64), skip_group_check=True,
                    )
                sle = slice(se * 512, (se + 1) * 512)
                slo = slice(so * 512, (so + 1) * 512)
                nc.vector.tensor_scalar_add(QT[0:H, sle], ps[0:H, :], b6_sb[0:H, 0:1])
                nc.vector.tensor_scalar_add(QT[H:P, slo], ps[H:P, :], b6_sb[H:P, 0:1])
                nc.sync.dma_start(QT[H:P, sle], QT[0:H, sle])
                nc.sync.dma_start(QT[0:H, slo], QT[H:P, slo])
                return first

            def epilogue(qw, outT):
                otsb = otp.tile([H + 1, 512], F32, name=f"otsb{qw}")
                nc.vector.tensor_copy(otsb[:], outT[:])
                nc.sync.dma_start(out2_h[:, qw * 512 : (qw + 1) * 512], otsb[:])

            # ---- emission ----
            xtiles = {sc: load_chunk(sc) for sc in range(NSC)}
            kv_mms(0, xtiles[0], 0, DC)
            v_trans(0, 0, 2)
            q_pass2(0, 1, xtiles[0], xtiles[1])

            # Drip schedule over half-0 iterations g = 2*p + qw:
            # kv chunk c in 4 pieces at g = 4c-4 .. 4c-1 (ready at pair 2c);
            # q chunks 2,3 (half 1) once chunk 3 has long arrived.
            drip = {}
            for c in range(1, NSC):
                drip.setdefault(4 * c - 4, []).append(
                    lambda c=c: kv_mms(c, xtiles[c], 0, 4))
                drip.setdefault(4 * c - 3, []).append(
                    lambda c=c: kv_mms(c, xtiles[c], 4, DC))
                drip.setdefault(4 * c - 2, []).append(
                    lambda c=c: v_trans(c, 0, 1))
                drip.setdefault(4 * c - 1, []).append(
                    lambda c=c: v_trans(c, 1, 2))
            drip.setdefault(28, []).append(
                lambda: q_pass2(2, 3, xtiles[2], xtiles[3]))

            pending = []

            for half in range(2):
                outTs = {
                    h2: pout.tile([H + 1, 512], F32, tag="outT",
                                  name=f"oT{half}_{h2}")
                    for h2 in range(2)
                }
                for p in range(NPAIR):
                    for h2 in range(2):
                        g = 2 * p + h2
                        tick = half * 0.40 + 0.01 * (g + 1)
                        tc.tile_set_cur_wait(tick)
                        qw = half * 2 + h2
                        qsl = slice(qw * 512, (qw + 1) * 512)
                        st = pst.tile([P, 1024], F32, tag="st", name=f"st{qw}_{p}")
                        ka = slice(2 * p * P, (2 * p + 1) * P)
                        kb = slice((2 * p + 1) * P, (2 * p + 2) * P)
                        nc.tensor.matmul(
                            st[:, 0:512], KT[0:H, ka], QT[0:H, qsl],
                            start=True, stop=True,
                        )
                        nc.tensor.matmul(
                            st[:, 512:1024], KT[H:P, kb], QT[H:P, qsl],
                            start=True, stop=True,
                        )
                        # DVE fast-exp: 1 of 3 iterations in the DMA-bound
                        # first half, every other one in the second half.
                        use_dve = (g % 3 == 2) if half == 0 else (g % 2 == 1)
                        if use_dve:
                            exi = expp.tile([P, 1024], I16, name="exi")
                            nc.vector.tensor_scalar(
                                exi[:], st[:], FE_SCALE, FE_BIAS,
                                op0=mybir.AluOpType.mult,
                                op1=mybir.AluOpType.add,
                            )
                            ex = exi[:].bitcast(BF16)
                        else:
                            exb = expp.tile([P, 1024], BF16, name="ex")
                            nc.scalar.activation(
                                exb[:], st[:], mybir.ActivationFunctionType.Exp
                            )
                            ex = exb[:]
                        if half == 0:
                            for fn in drip.get(g, []):
                                fn()

                        def pv(p=p, ex=ex, outT=outTs[h2], first=(p == 0),
                               last=(p == NPAIR - 1), qw=qw):
                            nc.tensor.matmul(
                                outT[:], Vn[:, 2 * p, :], ex[:, 0:512],
                                start=first, stop=False,
                            )
                            nc.tensor.matmul(
                                outT[:], Vn[:, 2 * p + 1, :], ex[:, 512:1024],
                                start=False, stop=last,
                            )
                            if last:
                                epilogue(qw, outT)
                        pending.append(pv)
                        while len(pending) > 2:
                            pending.pop(0)()
            tc.tile_set_cur_wait(0.9)
            while pending:
                pending.pop(0)()

    nc.compile()
    return nc


def _get_nc():
    if "nc" not in _NC_CACHE:
        _NC_CACHE["nc"] = build_core_graph()
    return _NC_CACHE["nc"]


def _make_in_maps(x, Wq, bq, Wk, bk, Wv, bv):
    x = np.asarray(x, dtype=np.float32)
    scale = np.float32(1.0 / np.sqrt(np.float32(H)))
    wq = np.asarray(Wq, np.float32) * scale
    wk = np.asarray(Wk, np.float32)
    wv = np.asarray(Wv, np.float32)
    wvk = np.concatenate([wv, wk], axis=1).astype(NP_BF16)
    wvk = np.ascontiguousarray(wvk.reshape(DC, P, P).transpose(1, 0, 2))
    wqp = np.ascontiguousarray(
        wq.astype(NP_BF16).reshape(DC, P, H).transpose(1, 0, 2)
    )
    b6 = np.zeros((P, 3), np.float32)
    b6[:, 0] = np.tile(np.asarray(bq, np.float32) * scale, 2)
    b6[H:P, 1] = np.asarray(bk, np.float32)
    b6[0:H, 2] = np.asarray(bv, np.float32)
    in_maps = []
    for core in range(8):
        b, half = divmod(core, 2)
        rolled = np.roll(x[b], -half * SQ, axis=0)
        xprep = np.ascontiguousarray(
            rolled.reshape(NSC, 512, DC, P).transpose(0, 3, 2, 1).astype(NP_BF16)
        )
        in_maps.append({"xt": xprep, "wvk": wvk, "wq": wqp, "b6": b6})
    return in_maps


def _gather(results):
    out = np.empty((4, S, H), dtype=np.float32)
    for core in range(8):
        b, half = divmod(core, 2)
        o2 = np.asarray(results[core]["out2"], np.float32)
        out[b, half * SQ : (half + 1) * SQ, :] = (o2[0:H] / o2[H : H + 1]).T
    return out


def run(trace=False, **inputs):
    """Run on hardware; returns (output, BassKernelResults)."""
    nc = _get_nc()
    in_maps = _make_in_maps(**inputs)
    res = run_bass_kernel_spmd(
        nc, in_maps, core_ids=list(range(8)), trace=trace
    )
    return _gather(res.results), res


def kernel(**inputs):
    out, _ = run(trace=False, **inputs)
    return out


# revision 19
# speedup vs baseline: 1.1351x; 1.1351x over previous
"""Single-head attention on 8 TRN2 NeuronCores (Bass/Tile).

Problem: x [4, 4096, 1024] f32; Wq/Wk/Wv [1024, 64]; bq/bk/bv [64].
  Q = x@Wq + bq; K = x@Wk + bk; V = x@Wv + bv
  out = softmax(Q K^T / 8) V        -> [4, 4096, 64]

Sharding: 8 cores = 4 batches x 2 query-halves. Every core gets its
batch's x pre-rotated (np.roll) on the host so its 2048 query rows are
always rows 0:2048 -> all cores run one identical static graph
(attention is permutation-invariant over keys). The host pre-permutes
x into the exact [chunk, partition, dchunk, s] SBUF layout (contiguous
8KB-per-partition DMA descriptors), pre-casts to bf16, and folds the
1/sqrt(64) score scale into Wq/bq. The device returns the UNNORMALIZED
attention output transposed ([64 h | 1 sums row] x 2048 queries); the
host divides by the sums row and transposes during the gather -- that
removes 16 PE transposes, the vector-engine normalize, and a
256B-run-strided output DMA from the device critical path.

Per-core kernel, v5:
- Scores for a key-tile PAIR run as two CONCURRENT row-tiled matmuls
  (h=64 contraction uses half the PE rows each; HW-measured ~2x).
  Even key tiles' K is DMA partition-shifted to rows 0:64 of KT;
  [Wq|Wq]-style duplication puts Q on both partition halves of QT.
- Q projections are col-tiled: one pass computes two 512-query chunks
  concurrently on the two PE column halves (64-wide Wq each), then two
  small DMAs mirror each chunk to the other partition half.
- Exp of the [128, 1024] score pair: scalar-engine ACTIVATE for most
  iterations; a share runs as Schraudolph fast-exp on the vector
  engine (TENSOR_SCALAR mult+add into int16, bit-viewed as bf16;
  HW-verified exact round-to-nearest, ~2% elementwise, scale-invariant
  under softmax).
- Software pipeline: iteration i emits scores(i)+exp(i); the PV
  accumulations of iteration i-2 follow, so neither exp engine ever
  waits on PV and the PE never waits on the current exp.
- Iteration order is PAIR-MAJOR over each half's two 512-query
  windows: pair p serves window 0 then window 1 before advancing, so
  the x-chunk DMA arrival window (first ~34us) overlaps 32 attention
  iterations instead of 16. KV chunk c drips in four pieces across
  iterations 4c-4..4c-1, finishing just before pair 2c needs it.
- V natural tiles via PE transposes; a ones column makes exp row-sums
  fall out of the PV matmul for free (row 64 of outT).
"""

import ml_dtypes
import numpy as np

import concourse.bass as bass
import concourse.mybir as mybir
import concourse.tile as tile
from concourse import bacc
from concourse.bass_utils import run_bass_kernel_spmd
from concourse.masks import make_identity

P = 128
D = 1024
DC = D // P  # 8 contraction chunks
S = 4096
SQ = 2048  # query rows per core
H = 64
NSC = S // 512  # 8 s-chunks of 512
NKT = S // P  # 32 key tiles of 128
NPAIR = NKT // 2  # 16 key-tile pairs
F32 = mybir.dt.float32
BF16 = mybir.dt.bfloat16
I16 = mybir.dt.int16
NP_BF16 = ml_dtypes.bfloat16

FE_SCALE = 128.0 / float(np.log(2.0))
FE_BIAS = 127.0 * 128.0 - 6.0

_NC_CACHE = {}


def build_core_graph():
    nc = bacc.Bacc(None, target_bir_lowering=False, debug=False)

    xt_h = nc.dram_tensor("xt", [NSC, P, DC, 512], BF16, kind="ExternalInput")
    wvk_h = nc.dram_tensor("wvk", [P, DC, P], BF16, kind="ExternalInput")
    wq_h = nc.dram_tensor("wq", [P, DC, H], BF16, kind="ExternalInput")
    b6_h = nc.dram_tensor("b6", [P, 3], F32, kind="ExternalInput")
    out2_h = nc.dram_tensor("out2", [H + 1, SQ], F32, kind="ExternalOutput")

    with tile.TileContext(nc) as tc:
        with (
            tc.tile_pool(name="const", bufs=1) as const,
            tc.tile_pool(name="xtp", bufs=8) as xtp,
            tc.tile_pool(name="expp", bufs=3) as expp,
            tc.tile_pool(name="otp", bufs=2) as otp,
            tc.tile_pool(name="pst", bufs=2, space="PSUM") as pst,
            tc.tile_pool(name="pkv", bufs=1, space="PSUM") as pkv,
            tc.tile_pool(name="pwork", bufs=1, space="PSUM") as pwork,
            tc.tile_pool(name="pout", bufs=2, space="PSUM") as pout,
        ):
            # ---- constants / persistent buffers ----
            wvk_sb = const.tile([P, DC, P], BF16, name="wvk_sb")
            wq_sb = const.tile([P, DC, H], BF16, name="wq_sb")
            b6_sb = const.tile([P, 3], F32, name="b6_sb")
            ident_b = const.tile([P, P], BF16, name="ident_b")
            KT = const.tile([P, S], BF16, name="KT")
            QT = const.tile([P, SQ], BF16, name="QT")
            # VT2: per key-tile PAIR, even kt's V^T on rows 0:64 and odd
            # kt's on 64:128 -> ONE [128,128] PE transpose yields BOTH
            # natural V tiles. VTst stages odd kts for the partition-shift
            # DMA (DVE cannot write across partition halves).
            VT2 = const.tile([P, S // 2], BF16, name="VT2")
            VTst = const.tile([H, NSC, 2, P], BF16, name="VTst")
            Vn = const.tile([P, NKT, H + 1], BF16, name="Vn")
            warm = const.tile([P, 3], F32, name="warm")

            nc.sync.dma_start(wvk_sb[:], wvk_h[:, :, :])
            nc.sync.dma_start(wq_sb[:], wq_h[:, :, :])
            nc.sync.dma_start(b6_sb[:], b6_h[:, :])
            make_identity(nc, ident_b[:])
            nc.gpsimd.memset(Vn[:, :, H : H + 1], 1.0)
            nc.scalar.activation(warm[:], b6_sb[:], mybir.ActivationFunctionType.Exp)
            # Dummy matmuls bridge the ~13us DMA lead-in so the HAM clock
            # gate stays released (1.2 -> 2.4 GHz) when real work arrives.
            # Rotate across the bank's four 128-col slices: consecutive
            # dummies hit disjoint psum regions, so they pipeline at the
            # ~56ns streaming rate instead of serializing on the WAW drain.
            wps = pkv.tile([P, 512], F32, tag="kv", name="warm_ps")
            for i in range(48):
                s = (i % 4) * P
                nc.tensor.matmul(
                    wps[:, s : s + P], ident_b[:], ident_b[:],
                    start=True, stop=True,
                )

            def load_chunk(sc):
                xtile = xtp.tile([P, DC, 512], BF16, name="xtile")
                nc.sync.dma_start(xtile[:], xt_h[sc])
                return xtile

            def kv_mms(sc, xtile, lo, hi):
                sl = slice(sc * 512, (sc + 1) * 512)
                if lo == 0:
                    kv_mms.ps[sc] = pkv.tile(
                        [P, 512], F32, tag="kv", name=f"kvps{sc}"
                    )
                ps = kv_mms.ps[sc]
                first = None
                for dc in range(lo, hi):
                    h = nc.tensor.matmul(
                        ps[:], wvk_sb[:, dc, :], xtile[:, dc, :],
                        start=(dc == 0), stop=(dc == DC - 1),
                    )
                    first = first or h
                if hi == DC:
                    psv = ps[0:H, :].rearrange("p (b k) -> p b k", k=P)
                    # even kts (psum col-blocks 0,2) -> VT2 rows 0:64
                    nc.vector.tensor_scalar_add(
                        VT2[0:H, sc * 256 : (sc + 1) * 256].rearrange(
                            "p (b k) -> p b k", k=P),
                        psv[:, 0::2], b6_sb[0:H, 2:3],
                    )
                    # odd kts -> staging, then partition-shift DMA to rows 64:128
                    nc.vector.tensor_scalar_add(
                        VTst[:, sc], psv[:, 1::2], b6_sb[0:H, 2:3]
                    )
                    nc.gpsimd.dma_start(
                        VT2[H:P, sc * 256 : (sc + 1) * 256].rearrange(
                            "p (b k) -> p b k", k=P),
                        VTst[:, sc],
                    )
                    nc.vector.tensor_scalar_add(KT[H:P, sl], ps[H:P, :], b6_sb[H:P, 1:2])
                    nc.gpsimd.dma_start(
                        KT[0:H, sl].rearrange("p (b k) -> p b k", k=P)[:, 0::2],
                        KT[H:P, sl].rearrange("p (b k) -> p b k", k=P)[:, 0::2],
                    )
                return first
            kv_mms.ps = {}

            def v_trans(sc, t0, t1):
                # one [128,128] transpose per key-tile PAIR
                first = None
                for t in range(t0, t1):
                    pr = sc * 2 + t
                    psl = slice(pr * P, (pr + 1) * P)
                    tp = pwork.tile([P, P], BF16, tag="work", name=f"vtp{pr}")
                    h = nc.tensor.transpose(tp[:], VT2[:, psl], ident_b[:])
                    first = first or h
                    nc.vector.tensor_copy(
                        Vn[:, 2 * pr : 2 * pr + 2, 0:H],
                        tp[:].rearrange("p (b k) -> p b k", k=H),
                    )
                return first

            def q_pass2(se, so, xte, xto):
                """Col-tiled [Wq] pass: chunk se -> psum rows 0:64, chunk so
                -> rows 64:128, concurrently; then mirror each to the other
                partition half of QT via DMA."""
                ps = pkv.tile([P, 512], F32, tag="kv", name=f"qps{se}")
                first = None
                for dc in range(DC):
                    h = nc.tensor.matmul(
                        ps[0:H, :], wq_sb[:, dc, :], xte[:, dc, :],
                        start=(dc == 0), stop=(dc == DC - 1),
                        tile_position=(0, 0), skip_group_check=True,
                    )
                    first = first or h
                    nc.tensor.matmul(
                        ps[H:P, :], wq_sb[:, dc, :], xto[:, dc, :],
                        start=(dc == 0), stop=(dc == DC - 1),
                        tile_position=(0, 64), skip_group_check=True,
                    )
                sle = slice(se * 512, (se + 1) * 512)
                slo = slice(so * 512, (so + 1) * 512)
                nc.vector.tensor_scalar_add(QT[0:H, sle], ps[0:H, :], b6_sb[0:H, 0:1])
                nc.vector.tensor_scalar_add(QT[H:P, slo], ps[H:P, :], b6_sb[H:P, 0:1])
                nc.gpsimd.dma_start(QT[H:P, sle], QT[0:H, sle])
                nc.gpsimd.dma_start(QT[0:H, slo], QT[H:P, slo])
                return first

            def epilogue(qw, outT):
                otsb = otp.tile([H + 1, 512], F32, name=f"otsb{qw}")
                nc.vector.tensor_copy(otsb[:], outT[:])
                nc.sync.dma_start(out2_h[:, qw * 512 : (qw + 1) * 512], otsb[:])

            # ---- emission ----
            xtiles = {sc: load_chunk(sc) for sc in range(NSC)}
            kv_mms(0, xtiles[0], 0, DC)
            v_trans(0, 0, 2)
            q_pass2(0, 1, xtiles[0], xtiles[1])

            # Drip schedule over half-0 iterations g = 2*p + qw:
            # kv chunk c in 4 pieces at g = 4c-4 .. 4c-1 (ready at pair 2c);
            # q chunks 2,3 (half 1) once chunk 3 has long arrived.
            drip = {}
            for c in range(1, NSC):
                drip.setdefault(4 * c - 4, []).append(
                    lambda c=c: kv_mms(c, xtiles[c], 0, 4))
                drip.setdefault(4 * c - 3, []).append(
                    lambda c=c: kv_mms(c, xtiles[c], 4, DC))
                drip.setdefault(4 * c - 2, []).append(
                    lambda c=c: v_trans(c, 0, 1))
                drip.setdefault(4 * c - 1, []).append(
                    lambda c=c: v_trans(c, 1, 2))
            drip.setdefault(28, []).append(
                lambda: q_pass2(2, 3, xtiles[2], xtiles[3]))

            pending = []

            for half in range(2):
                outTs = {
                    h2: pout.tile([H + 1, 512], F32, tag="outT",
                                  name=f"oT{half}_{h2}")
                    for h2 in range(2)
                }
                for p in range(NPAIR):
                    for h2 in range(2):
                        g = 2 * p + h2
                        tick = half * 0.40 + 0.01 * (g + 1)
                        tc.tile_set_cur_wait(tick)
                        qw = half * 2 + h2
                        qsl = slice(qw * 512, (qw + 1) * 512)
                        st = pst.tile([P, 1024], F32, tag="st", name=f"st{qw}_{p}")
                        ka = slice(2 * p * P, (2 * p + 1) * P)
                        kb = slice((2 * p + 1) * P, (2 * p + 2) * P)
                        nc.tensor.matmul(
                            st[:, 0:512], KT[0:H, ka], QT[0:H, qsl],
                            start=True, stop=True,
                        )
                        nc.tensor.matmul(
                            st[:, 512:1024], KT[H:P, kb], QT[H:P, qsl],
                            start=True, stop=True,
                        )
                        # DVE fast-exp: 1 of 3 iterations in the DMA-bound
                        # first half, every other one in the second half.
                        use_dve = (g % 3 == 2) if half == 0 else (g % 2 == 1)
                        if use_dve:
                            exi = expp.tile([P, 1024], I16, name="exi")
                            nc.vector.tensor_scalar(
                                exi[:], st[:], FE_SCALE, FE_BIAS,
                                op0=mybir.AluOpType.mult,
                                op1=mybir.AluOpType.add,
                            )
                            ex = exi[:].bitcast(BF16)
                        else:
                            exb = expp.tile([P, 1024], BF16, name="ex")
                            nc.scalar.activation(
                                exb[:], st[:], mybir.ActivationFunctionType.Exp
                            )
                            ex = exb[:]
                        if half == 0:
                            for fn in drip.get(g, []):
                                fn()

                        def pv(p=p, ex=ex, outT=outTs[h2], first=(p == 0),
                               last=(p == NPAIR - 1), qw=qw):
                            nc.tensor.matmul(
                                outT[:], Vn[:, 2 * p, :], ex[:, 0:512],
                                start=first, stop=False,
                            )
                            nc.tensor.matmul(
                                outT[:], Vn[:, 2 * p + 1, :], ex[:, 512:1024],
                                start=False, stop=last,
                            )
                            if last:
                                epilogue(qw, outT)
                        pending.append(pv)
                        while len(pending) > 2:
                            pending.pop(0)()
            tc.tile_set_cur_wait(0.9)
            while pending:
                pending.pop(0)()

    nc.compile()
    return nc


def _get_nc():
    if "nc" not in _NC_CACHE:
        _NC_CACHE["nc"] = build_core_graph()
    return _NC_CACHE["nc"]


def _make_in_maps(x, Wq, bq, Wk, bk, Wv, bv):
    x = np.asarray(x, dtype=np.float32)
    scale = np.float32(1.0 / np.sqrt(np.float32(H)))
    wq = np.asarray(Wq, np.float32) * scale
    wk = np.asarray(Wk, np.float32)
    wv = np.asarray(Wv, np.float32)
    wvk = np.concatenate([wv, wk], axis=1).astype(NP_BF16)
    wvk = np.ascontiguousarray(wvk.reshape(DC, P, P).transpose(1, 0, 2))
    wqp = np.ascontiguousarray(
        wq.astype(NP_BF16).reshape(DC, P, H).transpose(1, 0, 2)
    )
    b6 = np.zeros((P, 3), np.float32)
    b6[:, 0] = np.tile(np.asarray(bq, np.float32) * scale, 2)
    b6[H:P, 1] = np.asarray(bk, np.float32)
    b6[0:H, 2] = np.asarray(bv, np.float32)
    in_maps = []
    for core in range(8):
        b, half = divmod(core, 2)
        rolled = np.roll(x[b], -half * SQ, axis=0)
        xprep = np.ascontiguousarray(
            rolled.reshape(NSC, 512, DC, P).transpose(0, 3, 2, 1).astype(NP_BF16)
        )
        in_maps.append({"xt": xprep, "wvk": wvk, "wq": wqp, "b6": b6})
    return in_maps


def _gather(results):
    out = np.empty((4, S, H), dtype=np.float32)
    for core in range(8):
        b, half = divmod(core, 2)
        o2 = np.asarray(results[core]["out2"], np.float32)
        out[b, half * SQ : (half + 1) * SQ, :] = (o2[0:H] / o2[H : H + 1]).T
    return out


def run(trace=False, **inputs):
    """Run on hardware; returns (output, BassKernelResults)."""
    nc = _get_nc()
    in_maps = _make_in_maps(**inputs)
    res = run_bass_kernel_spmd(
        nc, in_maps, core_ids=list(range(8)), trace=trace
    )
    return _gather(res.results), res


def kernel(**inputs):
    out, _ = run(trace=False, **inputs)
    return out


# revision 20
# speedup vs baseline: 1.3506x; 1.1898x over previous
"""Single-head attention on 8 TRN2 NeuronCores (Bass/Tile). v5 (121.5us).

Problem: x [4, 4096, 1024] f32; Wq/Wk/Wv [1024, 64]; bq/bk/bv [64].
  Q = x@Wq + bq; K = x@Wk + bk; V = x@Wv + bv
  out = softmax(Q K^T / 8) V        -> [4, 4096, 64]

Sharding: 8 cores = 4 batches x 2 query-halves; x pre-rotated per core
(np.roll) so each core's 2048 query rows are rows 0:2048 (one SPMD
graph; softmax is permutation-invariant over keys). Host pre-permutes
x to the exact SBUF chunk layout (contiguous 8KB/partition DMA runs),
pre-casts bf16, folds the 1/8 score scale into Wq/bq. The device
returns UNNORMALIZED outT ([64 h | 1 sums row] x 2048 q); the host
divides by the sums row and transposes during the gather.

- Scores per key-tile PAIR: two CONCURRENT row-tiled matmuls (h=64
  contraction on each PE row half; ~2x measured). Even kts' K is DMA
  partition-shifted to KT rows 0:64; Q duplicated on both halves.
- Q projections col-tiled (two 512-q chunks on the PE column halves).
- Exp: scalar-engine ACTIVATE mostly; 1/3 (half 0) and 1/2 (half 1) of
  iterations use Schraudolph fast-exp on the vector engine
  (TENSOR_SCALAR f32->int16 mult+add, bitcast bf16; exact RTN on HW).
- Software pipeline: scores(i)+exp(i), then PV(i-2).
- PAIR-MAJOR iteration order over each half's two 512-query windows;
  KV chunk c drips in 4 pieces at iterations 4c-4..4c-1.
- V natural tiles via per-kt PE transposes + ones column (sums free).
"""

import ml_dtypes
import numpy as np

import concourse.bass as bass
import concourse.mybir as mybir
import concourse.tile as tile
from concourse import bacc
from concourse.bass_utils import run_bass_kernel_spmd
from concourse.masks import make_identity

P = 128
D = 1024
DC = D // P
S = 4096
SQ = 2048
H = 64
NSC = S // 512
NKT = S // P
NPAIR = NKT // 2
F32 = mybir.dt.float32
BF16 = mybir.dt.bfloat16
I16 = mybir.dt.int16
NP_BF16 = ml_dtypes.bfloat16

FE_SCALE = 128.0 / float(np.log(2.0))
FE_BIAS = 127.0 * 128.0 - 6.0

_NC_CACHE = {}


def build_core_graph():
    nc = bacc.Bacc(None, target_bir_lowering=False, debug=False)

    xt_h = nc.dram_tensor("xt", [NSC, P, DC, 512], BF16, kind="ExternalInput")
    wvk_h = nc.dram_tensor("wvk", [P, DC, P], BF16, kind="ExternalInput")
    wq_h = nc.dram_tensor("wq", [P, DC, H], BF16, kind="ExternalInput")
    b6_h = nc.dram_tensor("b6", [P, 3], F32, kind="ExternalInput")
    out2_h = nc.dram_tensor("out2", [H + 1, SQ], F32, kind="ExternalOutput")

    with tile.TileContext(nc) as tc:
        with (
            tc.tile_pool(name="const", bufs=1) as const,
            tc.tile_pool(name="xtp", bufs=8) as xtp,
            tc.tile_pool(name="expp", bufs=3) as expp,
            tc.tile_pool(name="otp", bufs=2) as otp,
            tc.tile_pool(name="pst", bufs=2, space="PSUM") as pst,
            tc.tile_pool(name="pkv", bufs=1, space="PSUM") as pkv,
            tc.tile_pool(name="pwork", bufs=1, space="PSUM") as pwork,
            tc.tile_pool(name="pout", bufs=2, space="PSUM") as pout,
        ):
            wvk_sb = const.tile([P, DC, P], BF16, name="wvk_sb")
            wq_sb = const.tile([P, DC, H], BF16, name="wq_sb")
            b6_sb = const.tile([P, 3], F32, name="b6_sb")
            ident_b = const.tile([P, P], BF16, name="ident_b")
            KT = const.tile([P, S], BF16, name="KT")
            QT = const.tile([P, SQ], BF16, name="QT")
            VT = const.tile([H, S], BF16, name="VT")
            Vn = const.tile([P, NKT, H + 1], BF16, name="Vn")
            warm = const.tile([P, 3], F32, name="warm")

            nc.sync.dma_start(wvk_sb[:], wvk_h[:, :, :])
            nc.sync.dma_start(wq_sb[:], wq_h[:, :, :])
            nc.sync.dma_start(b6_sb[:], b6_h[:, :])
            make_identity(nc, ident_b[:])
            nc.gpsimd.memset(Vn[:, :, H : H + 1], 1.0)
            nc.scalar.activation(warm[:], b6_sb[:], mybir.ActivationFunctionType.Exp)
            wps = pkv.tile([P, 512], F32, tag="kv", name="warm_ps")
            for _ in range(130):
                nc.tensor.matmul(
                    wps[:, 0:P], ident_b[:], ident_b[:], start=True, stop=True
                )

            def load_chunk(sc):
                xtile = xtp.tile([P, DC, 512], BF16, name="xtile")
                nc.sync.dma_start(xtile[:], xt_h[sc])
                return xtile

            def kv_mms(sc, xtile, lo, hi):
                sl = slice(sc * 512, (sc + 1) * 512)
                if lo == 0:
                    kv_mms.ps[sc] = pkv.tile(
                        [P, 512], F32, tag="kv", name=f"kvps{sc}"
                    )
                ps = kv_mms.ps[sc]
                for dc in range(lo, hi):
                    nc.tensor.matmul(
                        ps[:], wvk_sb[:, dc, :], xtile[:, dc, :],
                        start=(dc == 0), stop=(dc == DC - 1),
                    )
                if hi == DC:
                    nc.vector.tensor_scalar_add(VT[:, sl], ps[0:H, :], b6_sb[0:H, 2:3])
                    nc.vector.tensor_scalar_add(KT[H:P, sl], ps[H:P, :], b6_sb[H:P, 1:2])
                    nc.sync.dma_start(
                        KT[0:H, sl].rearrange("p (b k) -> p b k", k=P)[:, 0::2],
                        KT[H:P, sl].rearrange("p (b k) -> p b k", k=P)[:, 0::2],
                    )
            kv_mms.ps = {}

            def v_trans(sc, t0, t1):
                for t in range(t0, t1):
                    kt = sc * 4 + t
                    ksl = slice(kt * P, (kt + 1) * P)
                    tp = pwork.tile([P, H], BF16, tag="work", name=f"vtp{kt}")
                    nc.tensor.transpose(tp[:], VT[:, ksl], ident_b[0:H, 0:H])
                    nc.vector.tensor_copy(Vn[:, kt, 0:H], tp[:])

            def q_pass2(se, so, xte, xto):
                ps = pkv.tile([P, 512], F32, tag="kv", name=f"qps{se}")
                for dc in range(DC):
                    nc.tensor.matmul(
                        ps[0:H, :], wq_sb[:, dc, :], xte[:, dc, :],
                        start=(dc == 0), stop=(dc == DC - 1),
                        tile_position=(0, 0), skip_group_check=True,
                    )
                    nc.tensor.matmul(
                        ps[H:P, :], wq_sb[:, dc, :], xto[:, dc, :],
                        start=(dc == 0), stop=(dc == DC - 1),
                        tile_position=(0, 64), skip_group_check=True,
                    )
                sle = slice(se * 512, (se + 1) * 512)
                slo = slice(so * 512, (so + 1) * 512)
                nc.vector.tensor_scalar_add(QT[0:H, sle], ps[0:H, :], b6_sb[0:H, 0:1])
                nc.vector.tensor_scalar_add(QT[H:P, slo], ps[H:P, :], b6_sb[H:P, 0:1])
                nc.sync.dma_start(QT[H:P, sle], QT[0:H, sle])
                nc.sync.dma_start(QT[0:H, slo], QT[H:P, slo])

            def epilogue(qw, outT):
                otsb = otp.tile([H + 1, 512], F32, name=f"otsb{qw}")
                nc.vector.tensor_copy(otsb[:], outT[:])
                nc.sync.dma_start(out2_h[:, qw * 512 : (qw + 1) * 512], otsb[:])

            xtiles = {sc: load_chunk(sc) for sc in range(NSC)}
            kv_mms(0, xtiles[0], 0, DC)
            v_trans(0, 0, 4)
            q_pass2(0, 1, xtiles[0], xtiles[1])

            drip = {}
            for c in range(1, NSC):
                drip.setdefault(4 * c - 4, []).append(
                    lambda c=c: kv_mms(c, xtiles[c], 0, 4))
                drip.setdefault(4 * c - 3, []).append(
                    lambda c=c: kv_mms(c, xtiles[c], 4, DC))
                drip.setdefault(4 * c - 2, []).append(
                    lambda c=c: v_trans(c, 0, 2))
                drip.setdefault(4 * c - 1, []).append(
                    lambda c=c: v_trans(c, 2, 4))
            drip.setdefault(28, []).append(
                lambda: q_pass2(2, 3, xtiles[2], xtiles[3]))

            pending = []

            for half in range(2):
                outTs = {
                    h2: pout.tile([H + 1, 512], F32, tag="outT",
                                  name=f"oT{half}_{h2}")
                    for h2 in range(2)
                }
                for p in range(NPAIR):
                    for h2 in range(2):
                        g = 2 * p + h2
                        qw = half * 2 + h2
                        qsl = slice(qw * 512, (qw + 1) * 512)
                        st = pst.tile([P, 1024], F32, tag="st", name=f"st{qw}_{p}")
                        ka = slice(2 * p * P, (2 * p + 1) * P)
                        kb = slice((2 * p + 1) * P, (2 * p + 2) * P)
                        nc.tensor.matmul(
                            st[:, 0:512], KT[0:H, ka], QT[0:H, qsl],
                            start=True, stop=True,
                        )
                        nc.tensor.matmul(
                            st[:, 512:1024], KT[H:P, kb], QT[H:P, qsl],
                            start=True, stop=True,
                        )
                        use_dve = (g % 3 == 2) if half == 0 else (g % 2 == 1)
                        if use_dve:
                            exi = expp.tile([P, 1024], I16, name="exi")
                            nc.vector.tensor_scalar(
                                exi[:], st[:], FE_SCALE, FE_BIAS,
                                op0=mybir.AluOpType.mult,
                                op1=mybir.AluOpType.add,
                            )
                            ex = exi[:].bitcast(BF16)
                        else:
                            exb = expp.tile([P, 1024], BF16, name="ex")
                            nc.scalar.activation(
                                exb[:], st[:], mybir.ActivationFunctionType.Exp
                            )
                            ex = exb[:]
                        if half == 0:
                            for fn in drip.get(g, []):
                                fn()

                        def pv(p=p, ex=ex, outT=outTs[h2], first=(p == 0),
                               last=(p == NPAIR - 1), qw=qw):
                            nc.tensor.matmul(
                                outT[:], Vn[:, 2 * p, :], ex[:, 0:512],
                                start=first, stop=False,
                            )
                            nc.tensor.matmul(
                                outT[:], Vn[:, 2 * p + 1, :], ex[:, 512:1024],
                                start=False, stop=last,
                            )
                            if last:
                                epilogue(qw, outT)
                        pending.append(pv)
                        while len(pending) > 2:
                            pending.pop(0)()
            while pending:
                pending.pop(0)()

    nc.compile()
    return nc


def _get_nc():
    if "nc" not in _NC_CACHE:
        _NC_CACHE["nc"] = build_core_graph()
    return _NC_CACHE["nc"]


def _make_in_maps(x, Wq, bq, Wk, bk, Wv, bv):
    x = np.asarray(x, dtype=np.float32)
    scale = np.float32(1.0 / np.sqrt(np.float32(H)))
    wq = np.asarray(Wq, np.float32) * scale
    wk = np.asarray(Wk, np.float32)
    wv = np.asarray(Wv, np.float32)
    wvk = np.concatenate([wv, wk], axis=1).astype(NP_BF16)
    wvk = np.ascontiguousarray(wvk.reshape(DC, P, P).transpose(1, 0, 2))
    wqp = np.ascontiguousarray(
        wq.astype(NP_BF16).reshape(DC, P, H).transpose(1, 0, 2)
    )
    b6 = np.zeros((P, 3), np.float32)
    b6[:, 0] = np.tile(np.asarray(bq, np.float32) * scale, 2)
    b6[H:P, 1] = np.asarray(bk, np.float32)
    b6[0:H, 2] = np.asarray(bv, np.float32)
    in_maps = []
    for core in range(8):
        b, half = divmod(core, 2)
        rolled = np.roll(x[b], -half * SQ, axis=0)
        xprep = np.ascontiguousarray(
            rolled.reshape(NSC, 512, DC, P).transpose(0, 3, 2, 1).astype(NP_BF16)
        )
        in_maps.append({"xt": xprep, "wvk": wvk, "wq": wqp, "b6": b6})
    return in_maps


def _gather(results):
    out = np.empty((4, S, H), dtype=np.float32)
    for core in range(8):
        b, half = divmod(core, 2)
        o2 = np.asarray(results[core]["out2"], np.float32)
        out[b, half * SQ : (half + 1) * SQ, :] = (o2[0:H] / o2[H : H + 1]).T
    return out


def run(trace=False, **inputs):
    """Run on hardware; returns (output, BassKernelResults)."""
    nc = _get_nc()
    in_maps = _make_in_maps(**inputs)
    res = run_bass_kernel_spmd(
        nc, in_maps, core_ids=list(range(8)), trace=trace
    )
    return _gather(res.results), res


def kernel(**inputs):
    out, _ = run(trace=False, **inputs)
    return out
